# revision 1
# baseline (speedup 1.0000x reference)
"""Trainium2 Bass kernel for ConvSpikeEncoder (conv1d + BN-eval + LIF), v2.

Structure vs v1 baseline:
- 16 time-chunks (2 per core as chains A/B) instead of 8: halves the
  sequential step count per core (256 vs 480) at the cost of warmup
  (W ~ 136, ~90 spike flips expected => spk rel err ~7e-3 < 2e-2 gate).
- Batch columns split DVE/Pool per step: DVE handles cols [0, AC), Pool
  cols [AC, 64) as independent recurrences, both at pure busy rate via
  the 2-chain interleave (uA uB mA mB).
- Spike extraction moved to the otherwise-idle ACT engine:
  spk = Relu(Sign(mem - 1)) in fp16 (exact 0/1 values).
- Outputs: mem fp32, spk fp16, DMA'd per 32-step hist chunk; chain B's
  first 4 hist chunks (pure warmup) are not extracted or DMA'd.
- h' = conv + bias - 1 lives per-engine-layout: ACT copies conv PSUM to
  separate DVE-cols / Pool-cols SBUF tiles.
"""

import os
import sys

for _p in ("/opt/trn_rl_repo", "/root/.axon_site/_ro/trn_rl_repo"):
    if os.path.isdir(_p) and _p not in sys.path:
        sys.path.insert(0, _p)

import numpy as np

B, T, C_IN = 64, 512, 32
HID, TS, K = 128, 4, 3
C_OUT = HID * TS
N_CORES = 8
TAU = TS * T               # 2048 global steps
N_CH = 16                  # global time chunks (2 chains per core)
S = 240                    # computed steps per chain (10 hist chunks of 24)
TC = S // TS               # 60 conv t-steps per chain
JCH = 6                    # t-steps per conv chunk
NCONV = TC // JCH          # 10 conv chunks per chain == hist chunks
HSTEPS = 24                # recurrence steps per hist chunk
AC = 64                    # all batch cols on DVE (Pool lacks STT on HW)
PC = B - AC
B_SKIP = 4                 # chain-B hist chunks that are pure warmup

# real spans: chunk 0 gets S; chunks 1..15 split the rest (120*7 + 119*8),
# with W adjusted so each computed span starts on a conv t-step boundary.
_N_REST = TAU - S
_NK = [S] + [(_N_REST + i) // (N_CH - 1) for i in range(N_CH - 1)]
assert sum(_NK) == TAU

_T0 = [0]
for k in range(1, N_CH):
    _T0.append(_T0[-1] + _NK[k - 1])
# computed-span start, rounded UP to a multiple of TS so the real span
# [t0, t0+n) stays inside the computed window [GS, GS+S)
_GS = [0] + [-((-(t0 - (S - n))) // TS) * TS for t0, n in zip(_T0[1:], _NK[1:])]
_WK = [t0 - gs for t0, gs in zip(_T0, _GS)]
assert all(0 <= w <= S - 32 for w in _WK[1:]) and _WK[0] == 0
assert all(gs >= 0 and gs + S <= TAU for gs in _GS)
assert min(_WK[1:]) >= HSTEPS * B_SKIP  # skipped hist chunks are pure warmup

_CACHE = {}


def _build_program():
    from contextlib import ExitStack

    import concourse.bacc as bacc
    import concourse.tile as tile
    import concourse.mybir as mybir

    f32 = mybir.dt.float32
    f16 = mybir.dt.float16
    Alu = mybir.AluOpType
    Act = mybir.ActivationFunctionType

    nc = bacc.Bacc("TRN2", target_bir_lowering=False, debug=False,
                   enable_asserts=False, num_devices=N_CORES)

    # per-chain im2col'd x, streamed per conv chunk
    xa_d = nc.dram_tensor("xa", [98, TC * B], f32, kind="ExternalInput")
    xb_d = nc.dram_tensor("xb", [98, TC * B], f32, kind="ExternalInput")
    w_d = nc.dram_tensor("wts", [98, C_OUT], f32, kind="ExternalInput")
    beta_d = nc.dram_tensor("beta", [HID, 1], f32, kind="ExternalInput")
    # outputs: [hid, unit, sl, cols] per engine-part; chain A all 8 units,
    # chain B last 4. unit order: A0..A7, B4..B7.
    NU = NCONV + (NCONV - B_SKIP)   # 12 DMA'd units
    memd_o = nc.dram_tensor("mem_d", [HID, NU * HSTEPS * AC], f32,
                            kind="ExternalOutput")
    spkd_o = nc.dram_tensor("spk_d", [HID, NU * HSTEPS * AC], f16,
                            kind="ExternalOutput")

    with tile.TileContext(nc, num_cores=N_CORES) as tc:
        with ExitStack() as ctx:
            const = ctx.enter_context(tc.tile_pool(name="const", bufs=1))
            x_pool = ctx.enter_context(tc.tile_pool(name="x", bufs=6))
            hd_pool = ctx.enter_context(tc.tile_pool(name="hd", bufs=24))
            histd_pool = ctx.enter_context(tc.tile_pool(name="hsd", bufs=6))
            sgn_pool = ctx.enter_context(tc.tile_pool(name="sgn", bufs=2))
            spk_pool = ctx.enter_context(tc.tile_pool(name="spk", bufs=4))
            u_pool = ctx.enter_context(tc.tile_pool(name="u", bufs=6))
            psum = ctx.enter_context(tc.tile_pool(name="ps", bufs=8,
                                                  space="PSUM"))

            w_sb = const.tile([128, C_OUT], f32)
            nc.scalar.dma_start(w_sb[0:98, :], w_d[:, :])
            beta_sb = const.tile([HID, 1], f32)
            nc.gpsimd.dma_start(beta_sb[:, :], beta_d[:, :])
            zd_sb = const.tile([HID, AC], f32)
            nc.vector.memset(zd_sb[:, :], 0.0)

            x_d_ = {"A": xa_d, "B": xb_d}
            histd = {"A": [None] * NCONV, "B": [None] * NCONV}
            hd_t = {}
            out_off = {}  # (chain, ch) -> DMA unit index
            u_i = 0
            for ch in range(NCONV):
                if ch < B_SKIP:
                    out_off[("A", ch)] = ch
                else:
                    out_off[("A", ch)] = B_SKIP + 2 * (ch - B_SKIP)
                    out_off[("B", ch)] = B_SKIP + 2 * (ch - B_SKIP) + 1

            f32r = mybir.dt.float32r

            def emit_conv(ch):
                # conv for both chains: 4 psum groups each, copied to
                # per-engine h layouts. fp32r: 4x faster PE at FD=512.
                xts = {}
                for X in ("A", "B"):
                    xt = x_pool.tile([128, JCH * B], f32, name="xt")
                    cc = slice(ch * JCH * B, (ch + 1) * JCH * B)
                    nc.sync.dma_start(xt[0:98, :], x_d_[X][:, cc])
                    xts[X] = xt
                for g in range(TS):
                    for X in ("A", "B"):
                        xt = xts[X]
                        ps = psum.tile([128, JCH * B], f32, name="ps")
                        nc.tensor.matmul(
                            ps[:],
                            w_sb[0:98, g * 128:(g + 1) * 128],
                            xt[0:98, :],
                            start=True, stop=True)
                        hd = hd_pool.tile([128, JCH * B], f32, name="hd")
                        nc.scalar.copy(hd[:], ps[:])
                        hd_t[(X, g, ch)] = hd

            def emit_conv0():
                # chunk 0 in 4 sub-chunks of 2 t-steps: h for the first sls
                # lands after 1/4 of the PE work, shrinking the ramp
                xts = {}
                for X in ("A", "B"):
                    xt = x_pool.tile([128, JCH * B], f32, name="xt")
                    nc.sync.dma_start(xt[0:98, :], x_d_[X][:, 0:JCH * B])
                    xts[X] = xt
                SJ = 1
                for sub in range(JCH // SJ):
                    for g in range(TS):
                        for X in ("A", "B"):
                            xt = xts[X]
                            psf = psum.tile([128, JCH * B], f32, name="ps")
                            ps = psf[:, 0:SJ * B]
                            nc.tensor.matmul(
                                ps[:],
                                w_sb[0:98, g * 128:(g + 1) * 128],
                                xt[0:98, sub * SJ * B:(sub + 1) * SJ * B],
                                start=True, stop=True)
                            hd = hd_t[(X, g, 0)]
                            nc.scalar.copy(
                                hd[:, sub * SJ * B:(sub + 1) * SJ * B], ps[:])

            LOOKAHEAD = 2
            for X in ("A", "B"):
                for g in range(TS):
                    hd = hd_pool.tile([128, JCH * B], f32, name="hd")
                    hd_t[(X, g, 0)] = hd
            emit_conv0()
            emit_conv(1)

            for ch in range(NCONV):
                if ch + LOOKAHEAD < NCONV:
                    emit_conv(ch + LOOKAHEAD)

                htd = {X: histd_pool.tile([HID, HSTEPS * AC], f32,
                                          name="htd")
                       for X in ("A", "B")}
                for X in ("A", "B"):
                    histd[X][ch] = htd[X]

                def emit_mem_dma(q, eng=None):
                    for X in ("A", "B"):
                        if (X, ch) not in out_off:
                            continue
                        uo = out_off[(X, ch)]
                        for (ht, width, mem_o) in (
                                (htd[X], AC, memd_o),):
                            n = HSTEPS * width
                            hn = n // 4
                            lo = q * hn
                            (eng or nc.sync).dma_start(
                                mem_o[:, uo * n + lo:uo * n + lo + hn],
                                ht[:, lo:lo + hn])

                last = ch == NCONV - 1
                spk_half = {}
                for sl in range(HSTEPS):
                    if sl % (HSTEPS // 4) == 2 and sl > HSTEPS // 4:
                        emit_mem_dma(sl // (HSTEPS // 4) - 1)
                    if last and sl in (HSTEPS // 2 + 2, HSTEPS - 2):
                        # spikes in pieces on DVE + piece DMA as data lands
                        frac = 2 if sl == HSTEPS // 2 + 2 else 3
                        for X in ("A", "B"):
                            if (X, ch) not in out_off:
                                continue
                            uo2 = out_off[(X, ch)]
                            for (ht, width, eng, part) in (
                                    (htd[X], AC, nc.vector, "d"),):
                                n = HSTEPS * width
                                if frac == 2:
                                    sp = spk_pool.tile([HID, n], f16,
                                                       name="spl")
                                    spk_half[(X, part)] = sp
                                    lo, hi = 0, n // 2
                                else:
                                    sp = spk_half[(X, part)]
                                    lo, hi = n // 2, 3 * n // 4
                                    eng = nc.gpsimd
                                eng.tensor_scalar(sp[:, lo:hi],
                                                  ht[:, lo:hi], 1.0, None,
                                                  op0=Alu.is_gt)
                                nc.sync.dma_start(
                                    spkd_o[:, uo2 * n + lo:uo2 * n + hi],
                                    sp[:, lo:hi])
                    g = sl % TS
                    jc = sl // TS
                    # previous mem slices
                    def prev(hist_map, X, width):
                        if sl > 0:
                            t_ = hist_map[X][ch]
                            off = (sl - 1) * width
                        elif ch > 0:
                            t_ = hist_map[X][ch - 1]
                            off = (HSTEPS - 1) * width
                        else:
                            return None, 0
                        return t_, off

                    us_d = {}
                    for X in ("A", "B"):
                        mp, mo = prev(histd, X, AC)
                        src = zd_sb[:, 0:AC] if mp is None else mp[:, mo:mo + AC]
                        u = u_pool.tile([HID, AC], f32, name="u")
                        nc.vector.scalar_tensor_tensor(
                            u[:], src, 1.0,
                            hd_t[(X, g, ch)][:, jc * AC:(jc + 1) * AC],
                            op0=Alu.is_le, op1=Alu.add)
                        us_d[X] = (u, src)
                    for X in ("A", "B"):
                        u, src = us_d[X]
                        nc.vector.scalar_tensor_tensor(
                            htd[X][:, sl * AC:(sl + 1) * AC],
                            src, beta_sb[:, :], u[:],
                            op0=Alu.mult, op1=Alu.add)

                # spikes + DMA for units that carry real data
                emit_mem_dma(3, eng=nc.scalar if last else None)
                for X in ("A", "B"):
                    if (X, ch) not in out_off:
                        continue
                    uo = out_off[(X, ch)]
                    for (ht, width, mem_o, spk_o, eng) in (
                            (htd[X], AC, memd_o, spkd_o, nc.vector),):
                        n = HSTEPS * width
                        if last:
                            # tail: only the final quarter remains
                            sp = spk_half[(X, "d")]
                            eng.tensor_scalar(sp[:, 3 * n // 4:n],
                                              ht[:, 3 * n // 4:n], 1.0, None,
                                              op0=Alu.is_gt)
                            nc.gpsimd.dma_start(
                                spk_o[:, uo * n + 3 * n // 4:(uo + 1) * n],
                                sp[:, 3 * n // 4:n])
                        else:
                            sp = spk_pool.tile([HID, n], f16)
                            nc.gpsimd.tensor_scalar(sp[:], ht[:], 1.0, None,
                                                    op0=Alu.is_gt)
                            nc.gpsimd.dma_start(
                                spk_o[:, uo * n:(uo + 1) * n], sp[:])

    nc.compile()
    return nc


def _prep_inputs(x, conv_w, conv_b, bn_gamma, bn_beta, bn_mean, bn_var,
                 lif_beta):
    x = np.asarray(x, np.float32)
    conv_w = np.asarray(conv_w, np.float32)
    scale = (np.asarray(bn_gamma, np.float32)
             / np.sqrt(np.asarray(bn_var, np.float32) + 1e-5).astype(np.float32))
    w_f = conv_w * scale[:, None, None]                       # (512, 32, 3)
    b_f = ((np.asarray(conv_b, np.float32) - np.asarray(bn_mean, np.float32))
           * scale + np.asarray(bn_beta, np.float32))          # (512,)

    wts = np.zeros((98, C_OUT), np.float32)
    for k in range(K):
        wts[32 * k:32 * k + 32, :] = w_f[:, :, k].T
    wts[96, :] = b_f
    wts[97, :] = -1.0

    beta_h = np.clip(np.asarray(lif_beta, np.float32), 0.0, 1.0).reshape(HID, 1)

    xt = np.ascontiguousarray(x.transpose(2, 1, 0))            # (32, 512, 64)

    def im2col(gs):
        # computed g-steps [gs, gs+S) -> conv t-steps [gs/4, gs/4+TC)
        tv = gs // TS + np.arange(TC)
        valid = (tv >= 0) & (tv < T)
        xh = np.zeros((98, TC, B), np.float32)
        for k in range(K):
            tn = tv + k - 1
            ok = valid & (tn >= 0) & (tn < T)
            xh[32 * k:32 * k + 32, ok, :] = xt[:, tn[ok], :]
        xh[96, valid, :] = 1.0
        xh[97] = 1.0
        return np.ascontiguousarray(xh.reshape(98, TC * B))

    in_maps = []
    for c in range(N_CORES):
        in_maps.append({
            "xa": im2col(_GS[c]),
            "xb": im2col(_GS[c + 8]),
            "wts": wts,
            "beta": beta_h,
        })
    return in_maps


def kernel(x, conv_w, conv_b, bn_gamma, bn_beta, bn_mean, bn_var, lif_beta):
    from concourse.bass_utils import run_bass_kernel_spmd

    if "nc" not in _CACHE:
        _CACHE["nc"] = _build_program()
    nc = _CACHE["nc"]

    in_maps = _prep_inputs(x, conv_w, conv_b, bn_gamma, bn_beta,
                           bn_mean, bn_var, lif_beta)
    res = run_bass_kernel_spmd(nc, in_maps, core_ids=list(range(N_CORES)))
    _CACHE["last_result"] = res

    NU = NCONV + (NCONV - B_SKIP)
    spk = np.empty((TAU, B, HID), np.float32)
    mem = np.empty((TAU, B, HID), np.float32)

    def unit_index(ch):
        return B_SKIP + 2 * (ch - B_SKIP) + 1

    for c, r in enumerate(res.results):
        md = r["mem_d"].reshape(HID, NU, HSTEPS, AC)
        sd = r["spk_d"].astype(np.float32).reshape(HID, NU, HSTEPS, AC)

        def emit(k, units):
            # chunk k: computed steps [GS, GS+S) from the given unit list
            # (one unit per hist chunk, covering steps u*32..u*32+32)
            w, n, t0 = _WK[k], _NK[k], _T0[k]
            m_full = np.concatenate([md[:, u] for u in units], axis=1)
            s_full = np.concatenate([sd[:, u] for u in units], axis=1)
            base = S - len(units) * HSTEPS   # first step covered by units
            lo = w - base
            mem[t0:t0 + n] = m_full[:, lo:lo + n].transpose(1, 2, 0)
            spk[t0:t0 + n] = s_full[:, lo:lo + n].transpose(1, 2, 0)

        emit(c, [out_off_a(ch) for ch in range(NCONV)])
        emit(c + 8, [unit_index(ch) for ch in range(B_SKIP, NCONV)])
    return spk, mem


def out_off_a(ch):
    return ch if ch < B_SKIP else B_SKIP + 2 * (ch - B_SKIP)



# revision 2
# speedup vs baseline: 1.0513x; 1.0513x over previous
"""Trainium2 Bass kernel for ConvSpikeEncoder (conv1d + BN-eval + LIF), v2.

Structure vs v1 baseline:
- 16 time-chunks (2 per core as chains A/B) instead of 8: halves the
  sequential step count per core (256 vs 480) at the cost of warmup
  (W ~ 136, ~90 spike flips expected => spk rel err ~7e-3 < 2e-2 gate).
- Batch columns split DVE/Pool per step: DVE handles cols [0, AC), Pool
  cols [AC, 64) as independent recurrences, both at pure busy rate via
  the 2-chain interleave (uA uB mA mB).
- Spike extraction moved to the otherwise-idle ACT engine:
  spk = Relu(Sign(mem - 1)) in fp16 (exact 0/1 values).
- Outputs: mem fp32, spk fp16, DMA'd per 32-step hist chunk; chain B's
  first 4 hist chunks (pure warmup) are not extracted or DMA'd.
- h' = conv + bias - 1 lives per-engine-layout: ACT copies conv PSUM to
  separate DVE-cols / Pool-cols SBUF tiles.
"""

import os
import sys

for _p in ("/opt/trn_rl_repo", "/root/.axon_site/_ro/trn_rl_repo"):
    if os.path.isdir(_p) and _p not in sys.path:
        sys.path.insert(0, _p)

import numpy as np

B, T, C_IN = 64, 512, 32
HID, TS, K = 128, 4, 3
C_OUT = HID * TS
N_CORES = 8
TAU = TS * T               # 2048 global steps
N_CH = 16                  # global time chunks (2 chains per core)
S = 224                    # computed steps per chain (14 hist chunks of 16)
TC = S // TS               # 56 conv t-steps per chain
JCH = 4                    # t-steps per conv chunk
NCONV = TC // JCH          # 14 conv chunks per chain == hist chunks
HSTEPS = 16                # recurrence steps per hist chunk
AC = 64                    # all batch cols on DVE (Pool lacks STT on HW)
PC = B - AC
B_SKIP = 6                 # chain-B hist chunks that are pure warmup

# real spans: chunk 0 gets S; chunks 1..15 split the rest (120*7 + 119*8),
# with W adjusted so each computed span starts on a conv t-step boundary.
_N_REST = TAU - S
_NK = [S] + [(_N_REST + i) // (N_CH - 1) for i in range(N_CH - 1)]
assert sum(_NK) == TAU

_T0 = [0]
for k in range(1, N_CH):
    _T0.append(_T0[-1] + _NK[k - 1])
# computed-span start, rounded UP to a multiple of TS so the real span
# [t0, t0+n) stays inside the computed window [GS, GS+S)
_GS = [0] + [-((-(t0 - (S - n))) // TS) * TS for t0, n in zip(_T0[1:], _NK[1:])]
_WK = [t0 - gs for t0, gs in zip(_T0, _GS)]
assert all(0 <= w <= S - 32 for w in _WK[1:]) and _WK[0] == 0
assert all(gs >= 0 and gs + S <= TAU for gs in _GS)
assert min(_WK[1:]) >= HSTEPS * B_SKIP  # skipped hist chunks are pure warmup

_CACHE = {}


def _build_program():
    from contextlib import ExitStack

    import concourse.bacc as bacc
    import concourse.tile as tile
    import concourse.mybir as mybir

    f32 = mybir.dt.float32
    f16 = mybir.dt.float16
    Alu = mybir.AluOpType
    Act = mybir.ActivationFunctionType

    nc = bacc.Bacc("TRN2", target_bir_lowering=False, debug=False,
                   enable_asserts=False, num_devices=N_CORES)

    # per-chain im2col'd x, streamed per conv chunk
    xa_d = nc.dram_tensor("xa", [98, TC * B], f32, kind="ExternalInput")
    xb_d = nc.dram_tensor("xb", [98, TC * B], f32, kind="ExternalInput")
    w_d = nc.dram_tensor("wts", [98, C_OUT], f32, kind="ExternalInput")
    beta_d = nc.dram_tensor("beta", [HID, 1], f32, kind="ExternalInput")
    # outputs: [hid, unit, sl, cols] per engine-part; chain A all 8 units,
    # chain B last 4. unit order: A0..A7, B4..B7.
    NU = NCONV + (NCONV - B_SKIP)   # 12 DMA'd units
    memd_o = nc.dram_tensor("mem_d", [HID, NU * HSTEPS * AC], f32,
                            kind="ExternalOutput")
    spkd_o = nc.dram_tensor("spk_d", [HID, NU * HSTEPS * AC], f16,
                            kind="ExternalOutput")

    with tile.TileContext(nc, num_cores=N_CORES) as tc:
        with ExitStack() as ctx:
            const = ctx.enter_context(tc.tile_pool(name="const", bufs=1))
            x_pool = ctx.enter_context(tc.tile_pool(name="x", bufs=6))
            hd_pool = ctx.enter_context(tc.tile_pool(name="hd", bufs=24))
            histd_pool = ctx.enter_context(tc.tile_pool(name="hsd", bufs=6))
            sgn_pool = ctx.enter_context(tc.tile_pool(name="sgn", bufs=2))
            spk_pool = ctx.enter_context(tc.tile_pool(name="spk", bufs=4))
            u_pool = ctx.enter_context(tc.tile_pool(name="u", bufs=6))
            psum = ctx.enter_context(tc.tile_pool(name="ps", bufs=8,
                                                  space="PSUM"))

            w_sb = const.tile([128, C_OUT], f32)
            nc.scalar.dma_start(w_sb[0:98, :], w_d[:, :])
            beta_sb = const.tile([HID, 1], f32)
            nc.gpsimd.dma_start(beta_sb[:, :], beta_d[:, :])
            zd_sb = const.tile([HID, AC], f32)
            nc.vector.memset(zd_sb[:, :], 0.0)

            x_d_ = {"A": xa_d, "B": xb_d}
            histd = {"A": [None] * NCONV, "B": [None] * NCONV}
            hd_t = {}
            out_off = {}  # (chain, ch) -> DMA unit index
            u_i = 0
            for ch in range(NCONV):
                if ch < B_SKIP:
                    out_off[("A", ch)] = ch
                else:
                    out_off[("A", ch)] = B_SKIP + 2 * (ch - B_SKIP)
                    out_off[("B", ch)] = B_SKIP + 2 * (ch - B_SKIP) + 1

            f32r = mybir.dt.float32r

            def emit_conv(ch):
                # conv for both chains: 4 psum groups each, copied to
                # per-engine h layouts. fp32r: 4x faster PE at FD=512.
                xts = {}
                for X in ("A", "B"):
                    xt = x_pool.tile([128, JCH * B], f32, name="xt")
                    cc = slice(ch * JCH * B, (ch + 1) * JCH * B)
                    nc.sync.dma_start(xt[0:98, :], x_d_[X][:, cc])
                    xts[X] = xt
                for g in range(TS):
                    for X in ("A", "B"):
                        xt = xts[X]
                        ps = psum.tile([128, JCH * B], f32, name="ps")
                        nc.tensor.matmul(
                            ps[:],
                            w_sb[0:98, g * 128:(g + 1) * 128],
                            xt[0:98, :],
                            start=True, stop=True)
                        hd = hd_pool.tile([128, JCH * B], f32, name="hd")
                        nc.scalar.copy(hd[:], ps[:])
                        hd_t[(X, g, ch)] = hd

            def emit_conv0():
                # chunk 0 in 4 sub-chunks of 2 t-steps: h for the first sls
                # lands after 1/4 of the PE work, shrinking the ramp
                xts = {}
                for X in ("A", "B"):
                    xt = x_pool.tile([128, JCH * B], f32, name="xt")
                    nc.sync.dma_start(xt[0:98, :], x_d_[X][:, 0:JCH * B])
                    xts[X] = xt
                SJ = 1
                for sub in range(JCH // SJ):
                    for g in range(TS):
                        for X in ("A", "B"):
                            xt = xts[X]
                            psf = psum.tile([128, JCH * B], f32, name="ps")
                            ps = psf[:, 0:SJ * B]
                            nc.tensor.matmul(
                                ps[:],
                                w_sb[0:98, g * 128:(g + 1) * 128],
                                xt[0:98, sub * SJ * B:(sub + 1) * SJ * B],
                                start=True, stop=True)
                            hd = hd_t[(X, g, 0)]
                            nc.scalar.copy(
                                hd[:, sub * SJ * B:(sub + 1) * SJ * B], ps[:])

            LOOKAHEAD = 2
            for X in ("A", "B"):
                for g in range(TS):
                    hd = hd_pool.tile([128, JCH * B], f32, name="hd")
                    hd_t[(X, g, 0)] = hd
            emit_conv0()
            emit_conv(1)

            for ch in range(NCONV):
                if ch + LOOKAHEAD < NCONV:
                    emit_conv(ch + LOOKAHEAD)

                htd = {X: histd_pool.tile([HID, HSTEPS * AC], f32,
                                          name="htd")
                       for X in ("A", "B")}
                for X in ("A", "B"):
                    histd[X][ch] = htd[X]

                def emit_mem_dma(q, eng=None):
                    for X in ("A", "B"):
                        if (X, ch) not in out_off:
                            continue
                        uo = out_off[(X, ch)]
                        for (ht, width, mem_o) in (
                                (htd[X], AC, memd_o),):
                            n = HSTEPS * width
                            hn = n // 4
                            lo = q * hn
                            (eng or nc.sync).dma_start(
                                mem_o[:, uo * n + lo:uo * n + lo + hn],
                                ht[:, lo:lo + hn])

                last = ch == NCONV - 1
                spk_half = {}
                for sl in range(HSTEPS):
                    if sl % (HSTEPS // 4) == 2 and sl > HSTEPS // 4:
                        emit_mem_dma(sl // (HSTEPS // 4) - 1)
                    if last and sl in (HSTEPS // 2 + 2, HSTEPS - 2):
                        # spikes in pieces on DVE + piece DMA as data lands
                        frac = 2 if sl == HSTEPS // 2 + 2 else 3
                        for X in ("A", "B"):
                            if (X, ch) not in out_off:
                                continue
                            uo2 = out_off[(X, ch)]
                            for (ht, width, eng, part) in (
                                    (htd[X], AC, nc.vector, "d"),):
                                n = HSTEPS * width
                                if frac == 2:
                                    sp = spk_pool.tile([HID, n], f16,
                                                       name="spl")
                                    spk_half[(X, part)] = sp
                                    lo, hi = 0, n // 2
                                else:
                                    sp = spk_half[(X, part)]
                                    lo, hi = n // 2, 3 * n // 4
                                    eng = nc.gpsimd
                                eng.tensor_scalar(sp[:, lo:hi],
                                                  ht[:, lo:hi], 1.0, None,
                                                  op0=Alu.is_gt)
                                nc.sync.dma_start(
                                    spkd_o[:, uo2 * n + lo:uo2 * n + hi],
                                    sp[:, lo:hi])
                    g = sl % TS
                    jc = sl // TS
                    # previous mem slices
                    def prev(hist_map, X, width):
                        if sl > 0:
                            t_ = hist_map[X][ch]
                            off = (sl - 1) * width
                        elif ch > 0:
                            t_ = hist_map[X][ch - 1]
                            off = (HSTEPS - 1) * width
                        else:
                            return None, 0
                        return t_, off

                    us_d = {}
                    for X in ("A", "B"):
                        mp, mo = prev(histd, X, AC)
                        src = zd_sb[:, 0:AC] if mp is None else mp[:, mo:mo + AC]
                        u = u_pool.tile([HID, AC], f32, name="u")
                        nc.vector.scalar_tensor_tensor(
                            u[:], src, 1.0,
                            hd_t[(X, g, ch)][:, jc * AC:(jc + 1) * AC],
                            op0=Alu.is_le, op1=Alu.add)
                        us_d[X] = (u, src)
                    for X in ("A", "B"):
                        u, src = us_d[X]
                        nc.vector.scalar_tensor_tensor(
                            htd[X][:, sl * AC:(sl + 1) * AC],
                            src, beta_sb[:, :], u[:],
                            op0=Alu.mult, op1=Alu.add)

                # spikes + DMA for units that carry real data
                emit_mem_dma(3, eng=nc.scalar if last else None)
                for X in ("A", "B"):
                    if (X, ch) not in out_off:
                        continue
                    uo = out_off[(X, ch)]
                    for (ht, width, mem_o, spk_o, eng) in (
                            (htd[X], AC, memd_o, spkd_o, nc.vector),):
                        n = HSTEPS * width
                        if last:
                            # tail: only the final quarter remains
                            sp = spk_half[(X, "d")]
                            eng.tensor_scalar(sp[:, 3 * n // 4:n],
                                              ht[:, 3 * n // 4:n], 1.0, None,
                                              op0=Alu.is_gt)
                            nc.gpsimd.dma_start(
                                spk_o[:, uo * n + 3 * n // 4:(uo + 1) * n],
                                sp[:, 3 * n // 4:n])
                        else:
                            sp = spk_pool.tile([HID, n], f16)
                            nc.gpsimd.tensor_scalar(sp[:], ht[:], 1.0, None,
                                                    op0=Alu.is_gt)
                            nc.gpsimd.dma_start(
                                spk_o[:, uo * n:(uo + 1) * n], sp[:])

    nc.compile()
    return nc


def _prep_inputs(x, conv_w, conv_b, bn_gamma, bn_beta, bn_mean, bn_var,
                 lif_beta):
    x = np.asarray(x, np.float32)
    conv_w = np.asarray(conv_w, np.float32)
    scale = (np.asarray(bn_gamma, np.float32)
             / np.sqrt(np.asarray(bn_var, np.float32) + 1e-5).astype(np.float32))
    w_f = conv_w * scale[:, None, None]                       # (512, 32, 3)
    b_f = ((np.asarray(conv_b, np.float32) - np.asarray(bn_mean, np.float32))
           * scale + np.asarray(bn_beta, np.float32))          # (512,)

    wts = np.zeros((98, C_OUT), np.float32)
    for k in range(K):
        wts[32 * k:32 * k + 32, :] = w_f[:, :, k].T
    wts[96, :] = b_f
    wts[97, :] = -1.0

    beta_h = np.clip(np.asarray(lif_beta, np.float32), 0.0, 1.0).reshape(HID, 1)

    xt = np.ascontiguousarray(x.transpose(2, 1, 0))            # (32, 512, 64)

    def im2col(gs):
        # computed g-steps [gs, gs+S) -> conv t-steps [gs/4, gs/4+TC)
        tv = gs // TS + np.arange(TC)
        valid = (tv >= 0) & (tv < T)
        xh = np.zeros((98, TC, B), np.float32)
        for k in range(K):
            tn = tv + k - 1
            ok = valid & (tn >= 0) & (tn < T)
            xh[32 * k:32 * k + 32, ok, :] = xt[:, tn[ok], :]
        xh[96, valid, :] = 1.0
        xh[97] = 1.0
        return np.ascontiguousarray(xh.reshape(98, TC * B))

    in_maps = []
    for c in range(N_CORES):
        in_maps.append({
            "xa": im2col(_GS[c]),
            "xb": im2col(_GS[c + 8]),
            "wts": wts,
            "beta": beta_h,
        })
    return in_maps


def kernel(x, conv_w, conv_b, bn_gamma, bn_beta, bn_mean, bn_var, lif_beta):
    from concourse.bass_utils import run_bass_kernel_spmd

    if "nc" not in _CACHE:
        _CACHE["nc"] = _build_program()
    nc = _CACHE["nc"]

    in_maps = _prep_inputs(x, conv_w, conv_b, bn_gamma, bn_beta,
                           bn_mean, bn_var, lif_beta)
    res = run_bass_kernel_spmd(nc, in_maps, core_ids=list(range(N_CORES)))
    _CACHE["last_result"] = res

    NU = NCONV + (NCONV - B_SKIP)
    spk = np.empty((TAU, B, HID), np.float32)
    mem = np.empty((TAU, B, HID), np.float32)

    def unit_index(ch):
        return B_SKIP + 2 * (ch - B_SKIP) + 1

    for c, r in enumerate(res.results):
        md = r["mem_d"].reshape(HID, NU, HSTEPS, AC)
        sd = r["spk_d"].astype(np.float32).reshape(HID, NU, HSTEPS, AC)

        def emit(k, units):
            # chunk k: computed steps [GS, GS+S) from the given unit list
            # (one unit per hist chunk, covering steps u*32..u*32+32)
            w, n, t0 = _WK[k], _NK[k], _T0[k]
            m_full = np.concatenate([md[:, u] for u in units], axis=1)
            s_full = np.concatenate([sd[:, u] for u in units], axis=1)
            base = S - len(units) * HSTEPS   # first step covered by units
            lo = w - base
            mem[t0:t0 + n] = m_full[:, lo:lo + n].transpose(1, 2, 0)
            spk[t0:t0 + n] = s_full[:, lo:lo + n].transpose(1, 2, 0)

        emit(c, [out_off_a(ch) for ch in range(NCONV)])
        emit(c + 8, [unit_index(ch) for ch in range(B_SKIP, NCONV)])
    return spk, mem


def out_off_a(ch):
    return ch if ch < B_SKIP else B_SKIP + 2 * (ch - B_SKIP)



# revision 3
# speedup vs baseline: 1.0641x; 1.0121x over previous
"""Trainium2 Bass kernel for ConvSpikeEncoder (conv1d + BN-eval + LIF), v2.

Structure vs v1 baseline:
- 16 time-chunks (2 per core as chains A/B) instead of 8: halves the
  sequential step count per core (256 vs 480) at the cost of warmup
  (W ~ 136, ~90 spike flips expected => spk rel err ~7e-3 < 2e-2 gate).
- Batch columns split DVE/Pool per step: DVE handles cols [0, AC), Pool
  cols [AC, 64) as independent recurrences, both at pure busy rate via
  the 2-chain interleave (uA uB mA mB).
- Spike extraction moved to the otherwise-idle ACT engine:
  spk = Relu(Sign(mem - 1)) in fp16 (exact 0/1 values).
- Outputs: mem fp32, spk fp16, DMA'd per 32-step hist chunk; chain B's
  first 4 hist chunks (pure warmup) are not extracted or DMA'd.
- h' = conv + bias - 1 lives per-engine-layout: ACT copies conv PSUM to
  separate DVE-cols / Pool-cols SBUF tiles.
"""

import os
import sys

for _p in ("/opt/trn_rl_repo", "/root/.axon_site/_ro/trn_rl_repo"):
    if os.path.isdir(_p) and _p not in sys.path:
        sys.path.insert(0, _p)

import numpy as np

B, T, C_IN = 64, 512, 32
HID, TS, K = 128, 4, 3
C_OUT = HID * TS
N_CORES = 8
TAU = TS * T               # 2048 global steps
N_CH = 16                  # global time chunks (2 chains per core)
S = 224                    # computed steps per chain (14 hist chunks of 16)
TC = S // TS               # 56 conv t-steps per chain
JCH = 7                    # t-steps per conv chunk
NCONV = TC // JCH          # 8 conv chunks per chain == hist chunks
HSTEPS = 28                # recurrence steps per hist chunk
AC = 64                    # all batch cols on DVE (Pool lacks STT on HW)
PC = B - AC
B_SKIP = 3                 # chain-B hist chunks that are pure warmup

# real spans: chunk 0 gets S; chunks 1..15 split the rest (120*7 + 119*8),
# with W adjusted so each computed span starts on a conv t-step boundary.
_N_REST = TAU - S
_NK = [S] + [(_N_REST + i) // (N_CH - 1) for i in range(N_CH - 1)]
assert sum(_NK) == TAU

_T0 = [0]
for k in range(1, N_CH):
    _T0.append(_T0[-1] + _NK[k - 1])
# computed-span start, rounded UP to a multiple of TS so the real span
# [t0, t0+n) stays inside the computed window [GS, GS+S)
_GS = [0] + [-((-(t0 - (S - n))) // TS) * TS for t0, n in zip(_T0[1:], _NK[1:])]
_WK = [t0 - gs for t0, gs in zip(_T0, _GS)]
assert all(0 <= w <= S - 32 for w in _WK[1:]) and _WK[0] == 0
assert all(gs >= 0 and gs + S <= TAU for gs in _GS)
assert min(_WK[1:]) >= HSTEPS * B_SKIP  # skipped hist chunks are pure warmup

_CACHE = {}


def _build_program():
    from contextlib import ExitStack

    import concourse.bacc as bacc
    import concourse.tile as tile
    import concourse.mybir as mybir

    f32 = mybir.dt.float32
    f16 = mybir.dt.float16
    Alu = mybir.AluOpType
    Act = mybir.ActivationFunctionType

    nc = bacc.Bacc("TRN2", target_bir_lowering=False, debug=False,
                   enable_asserts=False, num_devices=N_CORES)

    # per-chain im2col'd x, streamed per conv chunk
    xa_d = nc.dram_tensor("xa", [98, TC * B], f32, kind="ExternalInput")
    xb_d = nc.dram_tensor("xb", [98, TC * B], f32, kind="ExternalInput")
    w_d = nc.dram_tensor("wts", [98, C_OUT], f32, kind="ExternalInput")
    beta_d = nc.dram_tensor("beta", [HID, 1], f32, kind="ExternalInput")
    # outputs: [hid, unit, sl, cols] per engine-part; chain A all 8 units,
    # chain B last 4. unit order: A0..A7, B4..B7.
    NU = NCONV + (NCONV - B_SKIP)   # 12 DMA'd units
    memd_o = nc.dram_tensor("mem_d", [HID, NU * HSTEPS * AC], f32,
                            kind="ExternalOutput")
    spkd_o = nc.dram_tensor("spk_d", [HID, NU * HSTEPS * AC], f16,
                            kind="ExternalOutput")

    with tile.TileContext(nc, num_cores=N_CORES) as tc:
        with ExitStack() as ctx:
            const = ctx.enter_context(tc.tile_pool(name="const", bufs=1))
            x_pool = ctx.enter_context(tc.tile_pool(name="x", bufs=6))
            hd_pool = ctx.enter_context(tc.tile_pool(name="hd", bufs=24))
            histd_pool = ctx.enter_context(tc.tile_pool(name="hsd", bufs=6))
            sgn_pool = ctx.enter_context(tc.tile_pool(name="sgn", bufs=2))
            spk_pool = ctx.enter_context(tc.tile_pool(name="spk", bufs=4))
            u_pool = ctx.enter_context(tc.tile_pool(name="u", bufs=6))
            psum = ctx.enter_context(tc.tile_pool(name="ps", bufs=8,
                                                  space="PSUM"))

            w_sb = const.tile([128, C_OUT], f32)
            nc.scalar.dma_start(w_sb[0:98, :], w_d[:, :])
            beta_sb = const.tile([HID, 1], f32)
            nc.gpsimd.dma_start(beta_sb[:, :], beta_d[:, :])
            zd_sb = const.tile([HID, AC], f32)
            nc.vector.memset(zd_sb[:, :], 0.0)

            x_d_ = {"A": xa_d, "B": xb_d}
            histd = {"A": [None] * NCONV, "B": [None] * NCONV}
            hd_t = {}
            out_off = {}  # (chain, ch) -> DMA unit index
            u_i = 0
            for ch in range(NCONV):
                if ch < B_SKIP:
                    out_off[("A", ch)] = ch
                else:
                    out_off[("A", ch)] = B_SKIP + 2 * (ch - B_SKIP)
                    out_off[("B", ch)] = B_SKIP + 2 * (ch - B_SKIP) + 1

            f32r = mybir.dt.float32r

            def emit_conv(ch):
                # conv for both chains: 4 psum groups each, copied to
                # per-engine h layouts. fp32r: 4x faster PE at FD=512.
                xts = {}
                for X in ("A", "B"):
                    xt = x_pool.tile([128, JCH * B], f32, name="xt")
                    cc = slice(ch * JCH * B, (ch + 1) * JCH * B)
                    nc.sync.dma_start(xt[0:98, :], x_d_[X][:, cc])
                    xts[X] = xt
                for g in range(TS):
                    for X in ("A", "B"):
                        xt = xts[X]
                        ps = psum.tile([128, JCH * B], f32, name="ps")
                        nc.tensor.matmul(
                            ps[:],
                            w_sb[0:98, g * 128:(g + 1) * 128],
                            xt[0:98, :],
                            start=True, stop=True)
                        hd = hd_pool.tile([128, JCH * B], f32, name="hd")
                        nc.scalar.copy(hd[:], ps[:])
                        hd_t[(X, g, ch)] = hd

            def emit_conv0():
                # chunk 0 in 4 sub-chunks of 2 t-steps: h for the first sls
                # lands after 1/4 of the PE work, shrinking the ramp
                xts = {}
                for X in ("A", "B"):
                    xt = x_pool.tile([128, JCH * B], f32, name="xt")
                    nc.sync.dma_start(xt[0:98, :], x_d_[X][:, 0:JCH * B])
                    xts[X] = xt
                SJ = 1
                for sub in range(JCH // SJ):
                    for g in range(TS):
                        for X in ("A", "B"):
                            xt = xts[X]
                            psf = psum.tile([128, JCH * B], f32, name="ps")
                            ps = psf[:, 0:SJ * B]
                            nc.tensor.matmul(
                                ps[:],
                                w_sb[0:98, g * 128:(g + 1) * 128],
                                xt[0:98, sub * SJ * B:(sub + 1) * SJ * B],
                                start=True, stop=True)
                            hd = hd_t[(X, g, 0)]
                            nc.scalar.copy(
                                hd[:, sub * SJ * B:(sub + 1) * SJ * B], ps[:])

            LOOKAHEAD = 2
            for X in ("A", "B"):
                for g in range(TS):
                    hd = hd_pool.tile([128, JCH * B], f32, name="hd")
                    hd_t[(X, g, 0)] = hd
            emit_conv0()
            emit_conv(1)

            for ch in range(NCONV):
                if ch + LOOKAHEAD < NCONV:
                    emit_conv(ch + LOOKAHEAD)

                htd = {X: histd_pool.tile([HID, HSTEPS * AC], f32,
                                          name="htd")
                       for X in ("A", "B")}
                for X in ("A", "B"):
                    histd[X][ch] = htd[X]

                def emit_mem_dma(q, eng=None):
                    for X in ("A", "B"):
                        if (X, ch) not in out_off:
                            continue
                        uo = out_off[(X, ch)]
                        for (ht, width, mem_o) in (
                                (htd[X], AC, memd_o),):
                            n = HSTEPS * width
                            hn = n // 4
                            lo = q * hn
                            (eng or nc.sync).dma_start(
                                mem_o[:, uo * n + lo:uo * n + lo + hn],
                                ht[:, lo:lo + hn])

                last = ch == NCONV - 1
                spk_half = {}
                for sl in range(HSTEPS):
                    if sl % (HSTEPS // 4) == 2 and sl > HSTEPS // 4:
                        emit_mem_dma(sl // (HSTEPS // 4) - 1)
                    if last and sl in (HSTEPS // 2 + 2, HSTEPS - 2):
                        # spikes in pieces on DVE + piece DMA as data lands
                        frac = 2 if sl == HSTEPS // 2 + 2 else 3
                        for X in ("A", "B"):
                            if (X, ch) not in out_off:
                                continue
                            uo2 = out_off[(X, ch)]
                            for (ht, width, eng, part) in (
                                    (htd[X], AC, nc.vector, "d"),):
                                n = HSTEPS * width
                                if frac == 2:
                                    sp = spk_pool.tile([HID, n], f16,
                                                       name="spl")
                                    spk_half[(X, part)] = sp
                                    lo, hi = 0, n // 2
                                else:
                                    sp = spk_half[(X, part)]
                                    lo, hi = n // 2, 3 * n // 4
                                    eng = nc.gpsimd
                                eng.tensor_scalar(sp[:, lo:hi],
                                                  ht[:, lo:hi], 1.0, None,
                                                  op0=Alu.is_gt)
                                nc.sync.dma_start(
                                    spkd_o[:, uo2 * n + lo:uo2 * n + hi],
                                    sp[:, lo:hi])
                    g = sl % TS
                    jc = sl // TS
                    # previous mem slices
                    def prev(hist_map, X, width):
                        if sl > 0:
                            t_ = hist_map[X][ch]
                            off = (sl - 1) * width
                        elif ch > 0:
                            t_ = hist_map[X][ch - 1]
                            off = (HSTEPS - 1) * width
                        else:
                            return None, 0
                        return t_, off

                    us_d = {}
                    for X in ("A", "B"):
                        mp, mo = prev(histd, X, AC)
                        src = zd_sb[:, 0:AC] if mp is None else mp[:, mo:mo + AC]
                        u = u_pool.tile([HID, AC], f32, name="u")
                        nc.vector.scalar_tensor_tensor(
                            u[:], src, 1.0,
                            hd_t[(X, g, ch)][:, jc * AC:(jc + 1) * AC],
                            op0=Alu.is_le, op1=Alu.add)
                        us_d[X] = (u, src)
                    for X in ("A", "B"):
                        u, src = us_d[X]
                        nc.vector.scalar_tensor_tensor(
                            htd[X][:, sl * AC:(sl + 1) * AC],
                            src, beta_sb[:, :], u[:],
                            op0=Alu.mult, op1=Alu.add)

                # spikes + DMA for units that carry real data
                emit_mem_dma(3, eng=nc.scalar if last else None)
                for X in ("A", "B"):
                    if (X, ch) not in out_off:
                        continue
                    uo = out_off[(X, ch)]
                    for (ht, width, mem_o, spk_o, eng) in (
                            (htd[X], AC, memd_o, spkd_o, nc.vector),):
                        n = HSTEPS * width
                        if last:
                            # tail: only the final quarter remains
                            sp = spk_half[(X, "d")]
                            eng.tensor_scalar(sp[:, 3 * n // 4:n],
                                              ht[:, 3 * n // 4:n], 1.0, None,
                                              op0=Alu.is_gt)
                            nc.gpsimd.dma_start(
                                spk_o[:, uo * n + 3 * n // 4:(uo + 1) * n],
                                sp[:, 3 * n // 4:n])
                        else:
                            sp = spk_pool.tile([HID, n], f16)
                            nc.gpsimd.tensor_scalar(sp[:], ht[:], 1.0, None,
                                                    op0=Alu.is_gt)
                            nc.gpsimd.dma_start(
                                spk_o[:, uo * n:(uo + 1) * n], sp[:])

    nc.compile()
    return nc


def _prep_inputs(x, conv_w, conv_b, bn_gamma, bn_beta, bn_mean, bn_var,
                 lif_beta):
    x = np.asarray(x, np.float32)
    conv_w = np.asarray(conv_w, np.float32)
    scale = (np.asarray(bn_gamma, np.float32)
             / np.sqrt(np.asarray(bn_var, np.float32) + 1e-5).astype(np.float32))
    w_f = conv_w * scale[:, None, None]                       # (512, 32, 3)
    b_f = ((np.asarray(conv_b, np.float32) - np.asarray(bn_mean, np.float32))
           * scale + np.asarray(bn_beta, np.float32))          # (512,)

    wts = np.zeros((98, C_OUT), np.float32)
    for k in range(K):
        wts[32 * k:32 * k + 32, :] = w_f[:, :, k].T
    wts[96, :] = b_f
    wts[97, :] = -1.0

    beta_h = np.clip(np.asarray(lif_beta, np.float32), 0.0, 1.0).reshape(HID, 1)

    xt = np.ascontiguousarray(x.transpose(2, 1, 0))            # (32, 512, 64)

    def im2col(gs):
        # computed g-steps [gs, gs+S) -> conv t-steps [gs/4, gs/4+TC)
        tv = gs // TS + np.arange(TC)
        valid = (tv >= 0) & (tv < T)
        xh = np.zeros((98, TC, B), np.float32)
        for k in range(K):
            tn = tv + k - 1
            ok = valid & (tn >= 0) & (tn < T)
            xh[32 * k:32 * k + 32, ok, :] = xt[:, tn[ok], :]
        xh[96, valid, :] = 1.0
        xh[97] = 1.0
        return np.ascontiguousarray(xh.reshape(98, TC * B))

    in_maps = []
    for c in range(N_CORES):
        in_maps.append({
            "xa": im2col(_GS[c]),
            "xb": im2col(_GS[c + 8]),
            "wts": wts,
            "beta": beta_h,
        })
    return in_maps


def kernel(x, conv_w, conv_b, bn_gamma, bn_beta, bn_mean, bn_var, lif_beta):
    from concourse.bass_utils import run_bass_kernel_spmd

    if "nc" not in _CACHE:
        _CACHE["nc"] = _build_program()
    nc = _CACHE["nc"]

    in_maps = _prep_inputs(x, conv_w, conv_b, bn_gamma, bn_beta,
                           bn_mean, bn_var, lif_beta)
    res = run_bass_kernel_spmd(nc, in_maps, core_ids=list(range(N_CORES)))
    _CACHE["last_result"] = res

    NU = NCONV + (NCONV - B_SKIP)
    spk = np.empty((TAU, B, HID), np.float32)
    mem = np.empty((TAU, B, HID), np.float32)

    def unit_index(ch):
        return B_SKIP + 2 * (ch - B_SKIP) + 1

    for c, r in enumerate(res.results):
        md = r["mem_d"].reshape(HID, NU, HSTEPS, AC)
        sd = r["spk_d"].astype(np.float32).reshape(HID, NU, HSTEPS, AC)

        def emit(k, units):
            # chunk k: computed steps [GS, GS+S) from the given unit list
            # (one unit per hist chunk, covering steps u*32..u*32+32)
            w, n, t0 = _WK[k], _NK[k], _T0[k]
            m_full = np.concatenate([md[:, u] for u in units], axis=1)
            s_full = np.concatenate([sd[:, u] for u in units], axis=1)
            base = S - len(units) * HSTEPS   # first step covered by units
            lo = w - base
            mem[t0:t0 + n] = m_full[:, lo:lo + n].transpose(1, 2, 0)
            spk[t0:t0 + n] = s_full[:, lo:lo + n].transpose(1, 2, 0)

        emit(c, [out_off_a(ch) for ch in range(NCONV)])
        emit(c + 8, [unit_index(ch) for ch in range(B_SKIP, NCONV)])
    return spk, mem


def out_off_a(ch):
    return ch if ch < B_SKIP else B_SKIP + 2 * (ch - B_SKIP)



# revision 5
# speedup vs baseline: 1.0860x; 1.0206x over previous
"""Trainium2 Bass kernel for ConvSpikeEncoder (conv1d + BN-eval + LIF), v2.

Structure vs v1 baseline:
- 16 time-chunks (2 per core as chains A/B) instead of 8: halves the
  sequential step count per core (256 vs 480) at the cost of warmup
  (W ~ 136, ~90 spike flips expected => spk rel err ~7e-3 < 2e-2 gate).
- Batch columns split DVE/Pool per step: DVE handles cols [0, AC), Pool
  cols [AC, 64) as independent recurrences, both at pure busy rate via
  the 2-chain interleave (uA uB mA mB).
- Spike extraction moved to the otherwise-idle ACT engine:
  spk = Relu(Sign(mem - 1)) in fp16 (exact 0/1 values).
- Outputs: mem fp32, spk fp16, DMA'd per 32-step hist chunk; chain B's
  first 4 hist chunks (pure warmup) are not extracted or DMA'd.
- h' = conv + bias - 1 lives per-engine-layout: ACT copies conv PSUM to
  separate DVE-cols / Pool-cols SBUF tiles.
"""

import os
import sys

for _p in ("/opt/trn_rl_repo", "/root/.axon_site/_ro/trn_rl_repo"):
    if os.path.isdir(_p) and _p not in sys.path:
        sys.path.insert(0, _p)

import numpy as np

B, T, C_IN = 64, 512, 32
HID, TS, K = 128, 4, 3
C_OUT = HID * TS
N_CORES = 8
TAU = TS * T               # 2048 global steps
N_CH = 16                  # global time chunks (2 chains per core)
S = 220                    # computed steps per chain (11 hist chunks of 20)
TC = S // TS               # 56 conv t-steps per chain
JCH = 5                    # t-steps per conv chunk
NCONV = TC // JCH          # 11 conv chunks per chain == hist chunks
HSTEPS = 20                # recurrence steps per hist chunk
AC = 64                    # all batch cols on DVE (Pool lacks STT on HW)
PC = B - AC
B_SKIP = 4                 # chain-B hist chunks that are pure warmup

# real spans: chunk 0 gets S; chunks 1..15 split the rest (120*7 + 119*8),
# with W adjusted so each computed span starts on a conv t-step boundary.
_N_REST = TAU - S
_NK = [S] + [(_N_REST + i) // (N_CH - 1) for i in range(N_CH - 1)]
assert sum(_NK) == TAU

_T0 = [0]
for k in range(1, N_CH):
    _T0.append(_T0[-1] + _NK[k - 1])
# computed-span start, rounded UP to a multiple of TS so the real span
# [t0, t0+n) stays inside the computed window [GS, GS+S)
_GS = [0] + [-((-(t0 - (S - n))) // TS) * TS for t0, n in zip(_T0[1:], _NK[1:])]
_WK = [t0 - gs for t0, gs in zip(_T0, _GS)]
assert all(0 <= w <= S - 32 for w in _WK[1:]) and _WK[0] == 0
assert all(gs >= 0 and gs + S <= TAU for gs in _GS)
assert min(_WK[1:]) >= HSTEPS * B_SKIP  # skipped hist chunks are pure warmup

_CACHE = {}


def _build_program():
    from contextlib import ExitStack

    import concourse.bacc as bacc
    import concourse.tile as tile
    import concourse.mybir as mybir

    f32 = mybir.dt.float32
    f16 = mybir.dt.float16
    Alu = mybir.AluOpType
    Act = mybir.ActivationFunctionType

    nc = bacc.Bacc("TRN2", target_bir_lowering=False, debug=False,
                   enable_asserts=False, num_devices=N_CORES)

    # per-chain im2col'd x, streamed per conv chunk
    xa_d = nc.dram_tensor("xa", [98, TC * B], f32, kind="ExternalInput")
    xb_d = nc.dram_tensor("xb", [98, TC * B], f32, kind="ExternalInput")
    w_d = nc.dram_tensor("wts", [98, C_OUT], f32, kind="ExternalInput")
    beta_d = nc.dram_tensor("beta", [HID, 1], f32, kind="ExternalInput")
    # outputs: [hid, unit, sl, cols] per engine-part; chain A all 8 units,
    # chain B last 4. unit order: A0..A7, B4..B7.
    NU = NCONV + (NCONV - B_SKIP)   # 12 DMA'd units
    memd_o = nc.dram_tensor("mem_d", [HID, NU * HSTEPS * AC], f32,
                            kind="ExternalOutput")
    spkd_o = nc.dram_tensor("spk_d", [HID, NU * HSTEPS * AC], f16,
                            kind="ExternalOutput")

    with tile.TileContext(nc, num_cores=N_CORES) as tc:
        with ExitStack() as ctx:
            const = ctx.enter_context(tc.tile_pool(name="const", bufs=1))
            x_pool = ctx.enter_context(tc.tile_pool(name="x", bufs=6))
            hd_pool = ctx.enter_context(tc.tile_pool(name="hd", bufs=24))
            histd_pool = ctx.enter_context(tc.tile_pool(name="hsd", bufs=6))
            sgn_pool = ctx.enter_context(tc.tile_pool(name="sgn", bufs=2))
            spk_pool = ctx.enter_context(tc.tile_pool(name="spk", bufs=4))
            u_pool = ctx.enter_context(tc.tile_pool(name="u", bufs=6))
            psum = ctx.enter_context(tc.tile_pool(name="ps", bufs=8,
                                                  space="PSUM"))

            w_sb = const.tile([128, C_OUT], f32)
            nc.scalar.dma_start(w_sb[0:98, :], w_d[:, :])
            beta_sb = const.tile([HID, 1], f32)
            nc.gpsimd.dma_start(beta_sb[:, :], beta_d[:, :])
            zd_sb = const.tile([HID, AC], f32)
            nc.vector.memset(zd_sb[:, :], 0.0)

            x_d_ = {"A": xa_d, "B": xb_d}
            histd = {"A": [None] * NCONV, "B": [None] * NCONV}
            hd_t = {}
            out_off = {}  # (chain, ch) -> DMA unit index
            u_i = 0
            for ch in range(NCONV):
                if ch < B_SKIP:
                    out_off[("A", ch)] = ch
                else:
                    out_off[("A", ch)] = B_SKIP + 2 * (ch - B_SKIP)
                    out_off[("B", ch)] = B_SKIP + 2 * (ch - B_SKIP) + 1

            f32r = mybir.dt.float32r

            def emit_conv(ch):
                # conv for both chains: 4 psum groups each, copied to
                # per-engine h layouts. fp32r: 4x faster PE at FD=512.
                xts = {}
                for X in ("A", "B"):
                    xt = x_pool.tile([128, JCH * B], f32, name="xt")
                    cc = slice(ch * JCH * B, (ch + 1) * JCH * B)
                    nc.sync.dma_start(xt[0:98, :], x_d_[X][:, cc])
                    xts[X] = xt
                for g in range(TS):
                    for X in ("A", "B"):
                        xt = xts[X]
                        ps = psum.tile([128, JCH * B], f32, name="ps")
                        nc.tensor.matmul(
                            ps[:],
                            w_sb[0:98, g * 128:(g + 1) * 128],
                            xt[0:98, :],
                            start=True, stop=True)
                        hd = hd_pool.tile([128, JCH * B], f32, name="hd")
                        nc.scalar.copy(hd[:], ps[:])
                        hd_t[(X, g, ch)] = hd

            def emit_conv0():
                # chunk 0 in 4 sub-chunks of 2 t-steps: h for the first sls
                # lands after 1/4 of the PE work, shrinking the ramp
                xts = {}
                for X in ("A", "B"):
                    xt = x_pool.tile([128, JCH * B], f32, name="xt")
                    nc.sync.dma_start(xt[0:98, :], x_d_[X][:, 0:JCH * B])
                    xts[X] = xt
                SJ = 1
                for sub in range(JCH // SJ):
                    for g in range(TS):
                        for X in ("A", "B"):
                            xt = xts[X]
                            psf = psum.tile([128, JCH * B], f32, name="ps")
                            ps = psf[:, 0:SJ * B]
                            nc.tensor.matmul(
                                ps[:],
                                w_sb[0:98, g * 128:(g + 1) * 128],
                                xt[0:98, sub * SJ * B:(sub + 1) * SJ * B],
                                start=True, stop=True)
                            hd = hd_t[(X, g, 0)]
                            nc.scalar.copy(
                                hd[:, sub * SJ * B:(sub + 1) * SJ * B], ps[:])

            LOOKAHEAD = 2
            for X in ("A", "B"):
                for g in range(TS):
                    hd = hd_pool.tile([128, JCH * B], f32, name="hd")
                    hd_t[(X, g, 0)] = hd
            emit_conv0()
            emit_conv(1)

            for ch in range(NCONV):
                if ch + LOOKAHEAD < NCONV:
                    emit_conv(ch + LOOKAHEAD)

                htd = {X: histd_pool.tile([HID, HSTEPS * AC], f32,
                                          name="htd")
                       for X in ("A", "B")}
                for X in ("A", "B"):
                    histd[X][ch] = htd[X]

                def emit_mem_dma(q, eng=None):
                    for X in ("A", "B"):
                        if (X, ch) not in out_off:
                            continue
                        uo = out_off[(X, ch)]
                        for (ht, width, mem_o) in (
                                (htd[X], AC, memd_o),):
                            n = HSTEPS * width
                            hn = n // 4
                            lo = q * hn
                            (eng or nc.sync).dma_start(
                                mem_o[:, uo * n + lo:uo * n + lo + hn],
                                ht[:, lo:lo + hn])

                last = ch == NCONV - 1
                spk_half = {}
                for sl in range(HSTEPS):
                    if sl % (HSTEPS // 4) == 2 and sl > HSTEPS // 4:
                        emit_mem_dma(sl // (HSTEPS // 4) - 1)
                    if last and sl in (HSTEPS // 2 + 2, HSTEPS - 2):
                        # spikes in pieces on DVE + piece DMA as data lands
                        frac = 2 if sl == HSTEPS // 2 + 2 else 3
                        for X in ("A", "B"):
                            if (X, ch) not in out_off:
                                continue
                            uo2 = out_off[(X, ch)]
                            for (ht, width, eng, part) in (
                                    (htd[X], AC, nc.gpsimd, "d"),):
                                n = HSTEPS * width
                                if frac == 2:
                                    sp = spk_pool.tile([HID, n], f16,
                                                       name="spl")
                                    spk_half[(X, part)] = sp
                                    lo, hi = 0, n // 2
                                else:
                                    sp = spk_half[(X, part)]
                                    lo, hi = n // 2, 3 * n // 4
                                    eng = nc.gpsimd
                                eng.tensor_scalar(sp[:, lo:hi],
                                                  ht[:, lo:hi], 1.0, None,
                                                  op0=Alu.is_gt)
                                nc.sync.dma_start(
                                    spkd_o[:, uo2 * n + lo:uo2 * n + hi],
                                    sp[:, lo:hi])
                    g = sl % TS
                    jc = sl // TS
                    # previous mem slices
                    def prev(hist_map, X, width):
                        if sl > 0:
                            t_ = hist_map[X][ch]
                            off = (sl - 1) * width
                        elif ch > 0:
                            t_ = hist_map[X][ch - 1]
                            off = (HSTEPS - 1) * width
                        else:
                            return None, 0
                        return t_, off

                    us_d = {}
                    for X in ("A", "B"):
                        mp, mo = prev(histd, X, AC)
                        src = zd_sb[:, 0:AC] if mp is None else mp[:, mo:mo + AC]
                        u = u_pool.tile([HID, AC], f32, name="u")
                        nc.vector.scalar_tensor_tensor(
                            u[:], src, 1.0,
                            hd_t[(X, g, ch)][:, jc * AC:(jc + 1) * AC],
                            op0=Alu.is_le, op1=Alu.add)
                        us_d[X] = (u, src)
                    for X in ("A", "B"):
                        u, src = us_d[X]
                        nc.vector.scalar_tensor_tensor(
                            htd[X][:, sl * AC:(sl + 1) * AC],
                            src, beta_sb[:, :], u[:],
                            op0=Alu.mult, op1=Alu.add)

                # spikes + DMA for units that carry real data
                emit_mem_dma(3, eng=nc.scalar if last else None)
                for X in ("A", "B"):
                    if (X, ch) not in out_off:
                        continue
                    uo = out_off[(X, ch)]
                    for (ht, width, mem_o, spk_o, eng) in (
                            (htd[X], AC, memd_o, spkd_o, nc.vector),):
                        n = HSTEPS * width
                        if last:
                            # tail: only the final quarter remains
                            sp = spk_half[(X, "d")]
                            eng.tensor_scalar(sp[:, 3 * n // 4:n],
                                              ht[:, 3 * n // 4:n], 1.0, None,
                                              op0=Alu.is_gt)
                            nc.gpsimd.dma_start(
                                spk_o[:, uo * n + 3 * n // 4:(uo + 1) * n],
                                sp[:, 3 * n // 4:n])
                        else:
                            sp = spk_pool.tile([HID, n], f16)
                            nc.gpsimd.tensor_scalar(sp[:], ht[:], 1.0, None,
                                                    op0=Alu.is_gt)
                            nc.gpsimd.dma_start(
                                spk_o[:, uo * n:(uo + 1) * n], sp[:])

    nc.compile()
    return nc


def _prep_inputs(x, conv_w, conv_b, bn_gamma, bn_beta, bn_mean, bn_var,
                 lif_beta):
    x = np.asarray(x, np.float32)
    conv_w = np.asarray(conv_w, np.float32)
    scale = (np.asarray(bn_gamma, np.float32)
             / np.sqrt(np.asarray(bn_var, np.float32) + 1e-5).astype(np.float32))
    w_f = conv_w * scale[:, None, None]                       # (512, 32, 3)
    b_f = ((np.asarray(conv_b, np.float32) - np.asarray(bn_mean, np.float32))
           * scale + np.asarray(bn_beta, np.float32))          # (512,)

    wts = np.zeros((98, C_OUT), np.float32)
    for k in range(K):
        wts[32 * k:32 * k + 32, :] = w_f[:, :, k].T
    wts[96, :] = b_f
    wts[97, :] = -1.0

    beta_h = np.clip(np.asarray(lif_beta, np.float32), 0.0, 1.0).reshape(HID, 1)

    xt = np.ascontiguousarray(x.transpose(2, 1, 0))            # (32, 512, 64)

    def im2col(gs):
        # computed g-steps [gs, gs+S) -> conv t-steps [gs/4, gs/4+TC)
        tv = gs // TS + np.arange(TC)
        valid = (tv >= 0) & (tv < T)
        xh = np.zeros((98, TC, B), np.float32)
        for k in range(K):
            tn = tv + k - 1
            ok = valid & (tn >= 0) & (tn < T)
            xh[32 * k:32 * k + 32, ok, :] = xt[:, tn[ok], :]
        xh[96, valid, :] = 1.0
        xh[97] = 1.0
        return np.ascontiguousarray(xh.reshape(98, TC * B))

    in_maps = []
    for c in range(N_CORES):
        in_maps.append({
            "xa": im2col(_GS[c]),
            "xb": im2col(_GS[c + 8]),
            "wts": wts,
            "beta": beta_h,
        })
    return in_maps


def kernel(x, conv_w, conv_b, bn_gamma, bn_beta, bn_mean, bn_var, lif_beta):
    from concourse.bass_utils import run_bass_kernel_spmd

    if "nc" not in _CACHE:
        _CACHE["nc"] = _build_program()
    nc = _CACHE["nc"]

    in_maps = _prep_inputs(x, conv_w, conv_b, bn_gamma, bn_beta,
                           bn_mean, bn_var, lif_beta)
    res = run_bass_kernel_spmd(nc, in_maps, core_ids=list(range(N_CORES)))
    _CACHE["last_result"] = res

    NU = NCONV + (NCONV - B_SKIP)
    spk = np.empty((TAU, B, HID), np.float32)
    mem = np.empty((TAU, B, HID), np.float32)

    def unit_index(ch):
        return B_SKIP + 2 * (ch - B_SKIP) + 1

    for c, r in enumerate(res.results):
        md = r["mem_d"].reshape(HID, NU, HSTEPS, AC)
        sd = r["spk_d"].astype(np.float32).reshape(HID, NU, HSTEPS, AC)

        def emit(k, units):
            # chunk k: computed steps [GS, GS+S) from the given unit list
            # (one unit per hist chunk, covering steps u*32..u*32+32)
            w, n, t0 = _WK[k], _NK[k], _T0[k]
            m_full = np.concatenate([md[:, u] for u in units], axis=1)
            s_full = np.concatenate([sd[:, u] for u in units], axis=1)
            base = S - len(units) * HSTEPS   # first step covered by units
            lo = w - base
            mem[t0:t0 + n] = m_full[:, lo:lo + n].transpose(1, 2, 0)
            spk[t0:t0 + n] = s_full[:, lo:lo + n].transpose(1, 2, 0)

        emit(c, [out_off_a(ch) for ch in range(NCONV)])
        emit(c + 8, [unit_index(ch) for ch in range(B_SKIP, NCONV)])
    return spk, mem


def out_off_a(ch):
    return ch if ch < B_SKIP else B_SKIP + 2 * (ch - B_SKIP)

